# revision 50
# baseline (speedup 1.0000x reference)
"""Causal self-attention (GQA + RoPE) Trainium2 kernel, v2.

Full-input contract: kernel(**inputs) takes the unsharded tensors and returns
the full [B, T, C] output. Internally shards over 8 NeuronCores as
(batch b in {0,1}) x (kv-head group g in {0..3}); each core computes the
attention output of its 4 query heads (one kv head) for its batch and the
partial out-projection against its 512 rows of Wo. The host sums the 4 group
partials per batch.

v2 structure (single TileContext scope, no mid-kernel barrier):
  - PSUM is one pool with shared tags across phases: "q" (4 bufs: Q-proj,
    score tiles, out-proj), "kv" (2: K/V-proj, attn@V accum), "sp" (2:
    V-transpose scratch, softmax denominators).
  - RoPE reads the projection PSUM directly on DVE, using cross-partition
    operand slices for the rotate-half term (PSUM in0 may have a different
    base partition than the SBUF in1 - verified on HW).
  - Causal F-restriction on score, exp, denominator and attn@V ops.
  - Softmax denominator via all-ones stationary matmul (PSUM-accumulated
    across k tiles), normalization as reciprocal*mul on DVE.
  - Phase B is software-pipelined: scores of tile i+1 are emitted between
    scores(i) and denom/attnV(i) so the mask->exp chain latency is hidden;
    the first two score groups of the next q-block are peeled ahead of the
    out-projection to cover the normalization latency.
  - Out-projection runs in bf16 (stationary ot tile, moving Wo), fp32 PSUM.
  - Weights/x are loaded with batched 3D-AP DMAs spread over the sync,
    gpsimd, scalar and tensor queues; y is stored as 1 MB row blocks.
"""

import sys

for _p in ("/opt/trn_rl_repo", "/root/.axon_site/_ro/trn_rl_repo"):
    if _p not in sys.path:
        sys.path.append(_p)

import numpy as np
from contextlib import ExitStack

import concourse.bass as bass
import concourse.bacc as bacc
import concourse.tile as tile
import concourse.mybir as mybir
from concourse.bass_utils import run_bass_kernel_spmd

F32 = mybir.dt.float32
F32R = mybir.dt.float32r
BF16 = mybir.dt.bfloat16
U16 = mybir.dt.uint16

B, T, C = 2, 2048, 2048
N_HEADS, N_KV_HEADS, HD = 16, 4, 128
G = N_HEADS // N_KV_HEADS  # heads per group = 4
GW = G * HD  # 512, per-group Q width / Wo row count
N_CORES = 8
TC = 512  # q-block width
NTC = T // TC  # 4
NCC = C // 128  # 16 contraction chunks
MASK_NEG = -1.0e30

_prog_cache = {}


def _build_program():
    nc = bacc.Bacc(
        "TRN2",
        target_bir_lowering=False,
        debug=False,
        enable_asserts=False,
        num_devices=N_CORES,
    )

    # host-pre-tiled layouts: contiguous >=2KB-per-partition lines so the
    # DMA descriptors stay fat (HBM small-descriptor penalty)
    xT = nc.dram_tensor("xT", [NTC * 4, 128, 4 * TC], F32, kind="ExternalInput").ap()
    wq = nc.dram_tensor("wq", [4, 128, 4 * GW], F32, kind="ExternalInput").ap()
    wk = nc.dram_tensor("wk", [128, NCC * HD], F32, kind="ExternalInput").ap()
    wv = nc.dram_tensor("wv", [128, NCC * HD], F32, kind="ExternalInput").ap()
    wo = nc.dram_tensor("wo", [GW, C], U16, kind="ExternalInput").ap()  # bf16 bits
    cos = nc.dram_tensor("cos", [HD, T], U16, kind="ExternalInput").ap()  # bf16 bits
    sin = nc.dram_tensor("sin", [HD, T], U16, kind="ExternalInput").ap()  # bf16 bits
    masks = nc.dram_tensor("masks", [128, TC], F32, kind="ExternalInput").ap()
    ident = nc.dram_tensor("ident", [128, 128], F32, kind="ExternalInput").ap()
    onesfull = nc.dram_tensor("onesfull", [128, 128], F32, kind="ExternalInput").ap()
    y = nc.dram_tensor("y", [T, C], F32, kind="ExternalOutput").ap()

    with tile.TileContext(nc) as tc, ExitStack() as ctx:
        sb = ctx.enter_context(tc.tile_pool(name="sb", bufs=1))
        xin = ctx.enter_context(tc.tile_pool(name="xin", bufs=2))
        rp = ctx.enter_context(tc.tile_pool(name="rp", bufs=2))
        ptp = ctx.enter_context(tc.tile_pool(name="ptp", bufs=6))
        nrm = ctx.enter_context(tc.tile_pool(name="nrm", bufs=2))
        otp = ctx.enter_context(tc.tile_pool(name="otp", bufs=2))
        ysb = ctx.enter_context(tc.tile_pool(name="ysb", bufs=2))
        ps = ctx.enter_context(tc.tile_pool(name="ps", bufs=1, space="PSUM"))

        # persistent activations
        qt_sb = sb.tile([128, G, T], F32R)  # [d, head, t]
        kt_sb = sb.tile([128, T], F32R)
        v_sb = sb.tile([128, T // 128, HD], F32R)  # [t-part, kt, d]

        # weights / tables. wq is split into four tiles (and wk/wv into
        # head/tail) so a matmul's DMA-completion dependency covers only
        # the chunks it actually reads.
        wq_t = [sb.tile([128, 4, GW], F32R, name=f"wq_t{i}") for i in range(4)]
        wk_a = sb.tile([128, 4, HD], F32R)
        wk_b = sb.tile([128, NCC - 4, HD], F32R)
        wv_a = sb.tile([128, 4, HD], F32R)
        wv_b = sb.tile([128, NCC - 4, HD], F32R)
        cos_sb = sb.tile([HD, T], BF16)
        sin_sb = sb.tile([HD, T], BF16)
        ident_sb = sb.tile([128, 128], F32)
        mask_sb = sb.tile([128, TC], F32)
        ones_sb = sb.tile([128, 128], F32R)
        wo_sb = sb.tile([128, G, C], BF16)

        # ---- weight prefetch. The two HW DMA rings (sync=SP, scalar=ACT)
        # serialize transfers per ring, so split the load: sync feeds x (+
        # the first K/V chunks), scalar feeds wq/wk/wv tails and tables.
        # wq group 0 goes as four per-ci chunks (fast first matmul), the
        # rest as contiguous 1MB blocks.
        # Keep the pre-loop DMA count at <=8: each matmul's completion wait
        # is on one of 8 shared lanes and counts every earlier-emitted DMA
        # on that lane, so extra prefetches here delay the FIRST matmul.
        for cc in range(4):
            nc.scalar.dma_start(
                wq_t[0][:, cc, :], wq[0, :, cc * GW : (cc + 1) * GW].bitcast(F32R)
            )
        nc.scalar.dma_start(wq_t[1][:].rearrange("p c j -> p (c j)"), wq[1].bitcast(F32R))
        nc.sync.dma_start(ident_sb[:], ident)

        # ---------------- phase A: projections + rope ----------------
        for tci in range(NTC):
            ts = slice(tci * TC, (tci + 1) * TC)
            qt_ps = [
                ps.tile([128, TC], F32, tag="q", bufs=4, name=f"qtps{tci}_{j}")
                for j in range(G)
            ]
            kt_ps = ps.tile([128, TC], F32, tag="kv", bufs=2, name=f"ktps{tci}")
            vt_ps = ps.tile([128, TC], F32, tag="kv", bufs=2, name=f"vtps{tci}")
            for cq in range(4):
                x_t = xin.tile([128, 4 * TC], F32R, tag="x", name=f"x{tci}_{cq}")
                blk = tci * 4 + cq
                # quads alternate sync / gpsimd rings (full SDMA transfer
                # bandwidth either way; keeps DMA issues out of the scalar
                # queue, which carries latency-critical copies)
                xq = nc.sync if cq % 2 == 0 else nc.gpsimd
                if tci == 0 and cq == 0:
                    # split the very first block so the first matmul can
                    # start after 512KB instead of 1MB
                    nc.sync.dma_start(
                        x_t[:, 0 : 2 * TC], xT[blk, :, 0 : 2 * TC].bitcast(F32R)
                    )
                    nc.sync.dma_start(
                        x_t[:, 2 * TC :], xT[blk, :, 2 * TC :].bitcast(F32R)
                    )
                    # first K/V weight chunks early on the sync ring
                    nc.sync.dma_start(
                        wk_a[:].rearrange("p c j -> p (c j)"),
                        wk[:, 0 : 4 * HD].bitcast(F32R),
                    )
                    nc.sync.dma_start(
                        wv_a[:].rearrange("p c j -> p (c j)"),
                        wv[:, 0 : 4 * HD].bitcast(F32R),
                    )
                    nc.sync.dma_start(
                        wk_b[:].rearrange("p c j -> p (c j)"),
                        wk[:, 4 * HD :].bitcast(F32R),
                    )
                    nc.sync.dma_start(
                        wv_b[:].rearrange("p c j -> p (c j)"),
                        wv[:, 4 * HD :].bitcast(F32R),
                    )
                else:
                    xq.dma_start(x_t[:], xT[blk].bitcast(F32R))
                if tci == 0 and cq == 1:
                    nc.scalar.dma_start(
                        wq_t[2][:].rearrange("p c j -> p (c j)"), wq[2].bitcast(F32R)
                    )
                    nc.sync.dma_start(
                        wq_t[3][:].rearrange("p c j -> p (c j)"), wq[3].bitcast(F32R)
                    )
                if tci == 0 and cq == 2:
                    nc.scalar.dma_start(cos_sb[:], cos.bitcast(BF16))
                    nc.scalar.dma_start(sin_sb[:], sin.bitcast(BF16))
                if tci == 0 and cq == 3:
                    nc.scalar.dma_start(mask_sb[:], masks)
                    nc.scalar.dma_start(ones_sb[:], onesfull.bitcast(F32R))
                    for h in range(G):
                        nc.scalar.dma_start(
                            wo_sb[:, h, :],
                            wo[h * 128 : (h + 1) * 128, :].bitcast(BF16),
                        )
                for sub in range(4):
                    ci = 4 * cq + sub
                    xs = x_t[:, sub * TC : (sub + 1) * TC]
                    st, sp = (ci == 0), (ci == NCC - 1)
                    wk_c = wk_a[:, ci, :] if ci < 4 else wk_b[:, ci - 4, :]
                    wv_c = wv_a[:, ci, :] if ci < 4 else wv_b[:, ci - 4, :]
                    for j in range(G):
                        nc.tensor.matmul(
                            qt_ps[j][:],
                            wq_t[ci // 4][:, ci % 4, j * HD : (j + 1) * HD],
                            xs,
                            start=st,
                            stop=sp,
                        )
                    nc.tensor.matmul(kt_ps[:], wk_c, xs, start=st, stop=sp)
                    nc.tensor.matmul(vt_ps[:], wv_c, xs, start=st, stop=sp)

            # rope: qt = q*cos + swap_half(q)*sin_signed. First evacuate all
            # six PSUM banks with fast aligned copies split across the
            # scalar and vector engines (so the next tci's matmuls aren't
            # gated on the slow rope chain), then build the half-swaps with
            # cross-partition scalar copies and finish the muls/add on DVE.
            q_raws = [
                rp.tile([128, TC], F32, tag=f"qraw{j}", name=f"qraw{tci}_{j}")
                for j in range(G)
            ]
            k_raw = rp.tile([128, TC], F32, tag="kraw", name=f"kraw{tci}")
            vt_f = rp.tile([128, TC], F32, tag="vtf", name=f"vtf{tci}")
            nc.scalar.copy(q_raws[0][:], qt_ps[0][:])
            nc.vector.tensor_copy(q_raws[1][:], qt_ps[1][:])
            nc.scalar.copy(q_raws[2][:], qt_ps[2][:])
            nc.vector.tensor_copy(q_raws[3][:], qt_ps[3][:])
            nc.scalar.copy(k_raw[:], kt_ps[:])
            nc.vector.tensor_copy(vt_f[:], vt_ps[:])

            # V: PE-transpose [d, t] -> [t, d] before the rope DVE work
            for s in range(TC // 128):
                kt_i = tci * (TC // 128) + s
                tp_t = ps.tile([128, TC], F32, tag="sp", bufs=2, name=f"tp{kt_i}")
                nc.tensor.transpose(
                    tp_t[:, 0:128], vt_f[:, s * 128 : (s + 1) * 128], ident_sb[:]
                )
                nc.scalar.copy(v_sb[:, kt_i, :], tp_t[:, 0:128])

            def make_rope(ts, q_raws, k_raw, tci):
                def rope(q_raw, dst, idx):
                    qsw = rp.tile([128, TC], F32, tag="qsw", name=f"qsw{idx}")
                    nc.scalar.copy(qsw[0:64, :], q_raw[64:128, :])
                    nc.scalar.copy(qsw[64:128, :], q_raw[0:64, :])
                    t1 = rp.tile([128, TC], F32, tag="t1", name=f"t1_{idx}")
                    nc.vector.tensor_mul(t1[:], q_raw[:], cos_sb[:, ts])
                    t2 = rp.tile([128, TC], F32, tag="t2", name=f"t2_{idx}")
                    nc.vector.tensor_mul(t2[:], qsw[:], sin_sb[:, ts])
                    nc.vector.tensor_add(dst, t1[:], t2[:])

                units = [
                    (lambda j=j: rope(q_raws[j], qt_sb[:, j, ts], f"{tci}_{j}"))
                    for j in range(G)
                ]
                units.append(lambda: rope(k_raw, kt_sb[:, ts], f"k{tci}"))
                return units

            rope_units = make_rope(ts, q_raws, k_raw, tci)
            if tci < NTC - 1:
                for u in rope_units:
                    u()
            else:
                # defer the last tci's rope (its outputs are only needed by
                # the later q-blocks) and spread it through qb0's pipeline
                # so the first q-block isn't queued behind 15 DVE ops
                deferred_rope = rope_units

        # -------- phase B: attention + out-projection per q-block --------
        # pending[(hg,)] closures emitted with a software-pipeline distance
        # of 2 between the score group S(i) and its denom/attnV group D(i).
        def make_block(qb, hg):
            # diag tiles first so PSUM start flag covers full columns
            kts = list(range(4 * qb, 4 * qb + 4)) + list(range(0, 4 * qb))
            n = len(kts)
            sb_ps = [
                ps.tile([128, TC], F32, tag="sp", bufs=2, name=f"sps{qb}_{hg}_{i}")
                for i in range(2)
            ]
            ot_ps = [
                ps.tile([128, TC], F32, tag="kv", bufs=2, name=f"otps{qb}_{hg}_{i}")
                for i in range(2)
            ]
            pts = {}

            def S(i):
                kt = kts[i]
                dj = kt - 4 * qb
                f0 = max(dj, 0) * 128
                cur = []
                for hh in range(2):
                    h = 2 * hg + hh
                    s_t = ps.tile(
                        [128, TC], F32, tag="q", bufs=4, name=f"st{qb}_{kt}_{h}"
                    )
                    if dj >= 0:
                        # pre-fill mask+zeros into PSUM off the critical
                        # chain; the score matmul accumulates on top
                        nc.vector.tensor_copy(
                            s_t[:, f0:TC], mask_sb[:, 0 : TC - f0]
                        )
                    nc.tensor.matmul(
                        s_t[:, f0:TC],
                        kt_sb[:, kt * 128 : (kt + 1) * 128],
                        qt_sb[:, h, qb * TC + f0 : (qb + 1) * TC],
                        start=(dj < 0),
                        stop=True,
                        skip_group_check=(dj >= 0),
                    )
                    pt = ptp.tile([128, TC], F32R, tag="pt", name=f"pt{qb}_{kt}_{h}")
                    nc.scalar.activation(
                        pt[:, f0:TC], s_t[:, f0:TC], mybir.ActivationFunctionType.Exp
                    )
                    cur.append(pt)
                pts[i] = (cur, f0)

            def D(i):
                kt = kts[i]
                cur, f0 = pts.pop(i)
                st_, sp_ = (i == 0), (i == n - 1)
                for hh in range(2):
                    nc.tensor.matmul(
                        sb_ps[hh][:, f0:TC],
                        ones_sb[:],
                        cur[hh][:, f0:TC],
                        start=st_,
                        stop=sp_,
                    )
                    nc.tensor.matmul(
                        ot_ps[hh][:, f0:TC],
                        v_sb[:, kt, :],
                        cur[hh][:, f0:TC],
                        start=st_,
                        stop=sp_,
                    )

            return S, D, n, sb_ps, ot_ps

        blocks = {}

        def emit_block(qb, hg, peeled, dve_filler=(), drain=True):
            S, D, n, sb_ps, ot_ps = blocks[(qb, hg)]
            filler = list(dve_filler)

            def fill():
                if filler:
                    filler.pop(0)()

            for i in range(peeled, n):
                S(i)
                if i >= 2:
                    D(i - 2)
                if i >= 1:
                    fill()
            D(n - 2)
            fill()
            D(n - 1)
            if drain:
                while filler:
                    filler.pop(0)()
            return sb_ps, ot_ps, filler

        # peel qb0's first two score groups so the PE rolls straight from
        # phase A into scores; the deferred rope rides along as filler
        blocks[(0, 0)] = make_block(0, 0)
        S0, _, _, _, _ = blocks[(0, 0)]
        S0(0)
        S0(1)

        peeled_next = 2
        for qb in range(NTC):
            blocks[(qb, 1)] = make_block(qb, 1)
            peeled = peeled_next

            # per-head normalized attention outputs (separate tiles so the
            # out-projection's h-loop tracks each head's norm individually)
            ot_t = [
                otp.tile([128, TC], BF16, tag=f"ot{h}", bufs=2, name=f"ot{qb}_{h}")
                for h in range(G)
            ]

            def norm_ops(hg, sbp, otps):
                rfs = {}

                def mk(hh):
                    h = 2 * hg + hh

                    def op_r():
                        r_f = nrm.tile([128, TC], F32, tag="rf", name=f"rf{qb}_{h}")
                        nc.vector.reciprocal_approx_fast(r_f[:], sbp[hh][:])
                        rfs[h] = r_f

                    def op_m():
                        nc.vector.tensor_mul(ot_t[h][:], otps[hh][:], rfs[h][:])

                    return [op_r, op_m]

                return mk(0) + mk(1)

            hg0_filler = list(deferred_rope) if qb == 0 else []
            sb0, ot0, left = emit_block(qb, 0, peeled, hg0_filler, drain=False)
            # hg0's normalization (and any leftover deferred rope) rides
            # along inside hg1's pipeline
            sb1, ot1, _ = emit_block(qb, 1, 0, left + norm_ops(0, sb0, ot0))

            # peel the first score groups of the next q-block ahead of the
            # normalization + out-projection; its first denominator/attnV
            # groups and two more score groups are interleaved into the
            # out-projection stream below
            if qb + 1 < NTC:
                blocks[(qb + 1, 0)] = make_block(qb + 1, 0)
                Sn, Dn, _, _, _ = blocks[(qb + 1, 0)]
                Sn(0)
                Sn(1)
                peeled_next = 4
            else:
                Sn = Dn = None
            for op in norm_ops(1, sb1, ot1):
                op()

            # out-projection for this q-block, stored as 512KB half-row
            # blocks alternating between the two DMA rings
            gi = 0
            for tl in range(TC // 128):
                tsub = qb * (TC // 128) + tl
                last = tsub == T // 128 - 1
                for half in range(2):
                    y_sb = ysb.tile([128, 2 * TC], F32, tag="y", name=f"ysb{tsub}_{half}")
                    for sub in range(2):
                        cc = 2 * half + sub
                        y_ps = ps.tile(
                            [128, TC], F32, tag="q", bufs=4, name=f"yps{tsub}_{cc}"
                        )
                        for h in range(G):
                            nc.tensor.matmul(
                                y_ps[:],
                                ot_t[h][:, tl * 128 : (tl + 1) * 128],
                                wo_sb[:, h, cc * TC : (cc + 1) * TC],
                                start=(h == 0),
                                stop=(h == G - 1),
                            )
                        nc.vector.tensor_copy(y_sb[:, sub * TC : (sub + 1) * TC], y_ps[:])
                        if last:
                            yq = (nc.sync, nc.scalar)[sub]
                            yq.dma_start(
                                y[
                                    tsub * 128 : (tsub + 1) * 128,
                                    cc * TC : (cc + 1) * TC,
                                ],
                                y_sb[:, sub * TC : (sub + 1) * TC],
                            )
                        if Sn is not None:
                            if gi == 3:
                                Dn(0)
                            elif gi == 7:
                                Dn(1)
                            elif gi == 11:
                                Sn(2)
                            elif gi == 13:
                                Sn(3)
                        gi += 1
                    if not last:
                        yq = nc.sync if half == 0 else nc.scalar
                        yq.dma_start(
                            y[
                                tsub * 128 : (tsub + 1) * 128,
                                half * 2 * TC : (half + 1) * 2 * TC,
                            ],
                            y_sb[:],
                        )

    nc.compile()
    return nc


def _rope_tables():
    theta = 1.0 / (10000.0 ** (np.arange(0, HD, 2, dtype=np.float32) / HD))
    freqs = np.arange(T, dtype=np.float32)[:, None] * theta[None, :]  # [T, 64]
    cos = np.concatenate([np.cos(freqs), np.cos(freqs)], axis=-1)  # [T, 128]
    sin = np.concatenate([np.sin(freqs), np.sin(freqs)], axis=-1)
    cosT = np.ascontiguousarray(cos.T).astype(np.float32)  # [128, T]
    sinT = np.ascontiguousarray(sin.T).astype(np.float32)
    sign = np.where(np.arange(HD) < HD // 2, np.float32(-1.0), np.float32(1.0))[:, None]
    sinT_signed = (sinT * sign).astype(np.float32)
    return cosT, sinT_signed


def _masks():
    # [128, TC]: first 128 cols = causal triangle bias (0 valid / -1e30
    # masked), rest zeros; copied into PSUM ahead of diagonal score matmuls
    p = np.arange(128)[:, None]
    f = np.arange(128)[None, :]
    tri = np.where(p <= f, 0.0, MASK_NEG).astype(np.float32)
    out = np.zeros((128, TC), dtype=np.float32)
    out[:, :128] = tri
    return out


def _to_bf16_bits(a):
    # round-to-nearest-even fp32 -> bf16, returned as uint16 bit pattern
    u = np.asarray(a, dtype=np.float32).view(np.uint32).astype(np.uint64)
    rounded = (u + 0x7FFF + ((u >> 16) & 1)) >> 16
    return rounded.astype(np.uint16)


def make_in_maps(x, Wq, Wk, Wv, Wo):
    x = np.asarray(x, dtype=np.float32)
    Wq = np.asarray(Wq, dtype=np.float32)
    Wk = np.asarray(Wk, dtype=np.float32)
    Wv = np.asarray(Wv, dtype=np.float32)
    Wo = np.asarray(Wo, dtype=np.float32)

    cosT, sinT = _rope_tables()
    masks = _masks()
    qscale = np.float32(1.0 / np.sqrt(HD))
    ident = np.eye(128, dtype=np.float32)
    onesfull = np.ones((128, 128), dtype=np.float32)

    def tile_x(xb):
        # xT [C, T] -> [tci*4+q, 128, 4*512] with 8KB contiguous lines
        xT_ = xb.T.reshape(4, 4, 128, 4, 512)  # [q, cc, p, tci, tt]
        return np.ascontiguousarray(xT_.transpose(3, 0, 2, 1, 4)).reshape(16, 128, 2048)

    def tile_wq(w):
        w_ = w.reshape(4, 4, 128, GW)  # [g, cc, p, j]
        return np.ascontiguousarray(w_.transpose(0, 2, 1, 3)).reshape(4, 128, 4 * GW)

    def tile_wkv(w):
        w_ = w.reshape(NCC, 128, HD)  # [c, p, j]
        return np.ascontiguousarray(w_.transpose(1, 0, 2)).reshape(128, NCC * HD)

    in_maps = []
    for c in range(N_CORES):
        b, g = divmod(c, N_KV_HEADS)
        in_maps.append(
            {
                "xT": tile_x(x[b]),
                "wq": tile_wq(
                    np.ascontiguousarray(Wq[:, g * GW : (g + 1) * GW]) * qscale
                ),
                "wk": tile_wkv(np.ascontiguousarray(Wk[:, g * HD : (g + 1) * HD])),
                "wv": tile_wkv(np.ascontiguousarray(Wv[:, g * HD : (g + 1) * HD])),
                "wo": _to_bf16_bits(Wo[g * GW : (g + 1) * GW, :]),
                "cos": _to_bf16_bits(cosT),
                "sin": _to_bf16_bits(sinT),
                "masks": masks,
                "ident": ident,
                "onesfull": onesfull,
            }
        )
    return in_maps


def kernel(x, Wq, Wk, Wv, Wo):
    if "nc" not in _prog_cache:
        _prog_cache["nc"] = _build_program()
    nc = _prog_cache["nc"]

    in_maps = make_in_maps(x, Wq, Wk, Wv, Wo)
    res = run_bass_kernel_spmd(nc, in_maps, list(range(N_CORES)))
    _prog_cache["last_results"] = res

    out = np.zeros((B, T, C), dtype=np.float32)
    for c in range(N_CORES):
        b = c // N_KV_HEADS
        out[b] += res.results[c]["y"]
    return out


# revision 52
# speedup vs baseline: 1.0661x; 1.0661x over previous
"""Causal self-attention (GQA + RoPE) Trainium2 kernel, v2.

Full-input contract: kernel(**inputs) takes the unsharded tensors and returns
the full [B, T, C] output. Internally shards over 8 NeuronCores as
(batch b in {0,1}) x (kv-head group g in {0..3}); each core computes the
attention output of its 4 query heads (one kv head) for its batch and the
partial out-projection against its 512 rows of Wo. The host sums the 4 group
partials per batch.

v2 structure (single TileContext scope, no mid-kernel barrier):
  - PSUM is one pool with shared tags across phases: "q" (4 bufs: Q-proj,
    score tiles, out-proj), "kv" (2: K/V-proj, attn@V accum), "sp" (2:
    V-transpose scratch, softmax denominators).
  - RoPE reads the projection PSUM directly on DVE, using cross-partition
    operand slices for the rotate-half term (PSUM in0 may have a different
    base partition than the SBUF in1 - verified on HW).
  - Causal F-restriction on score, exp, denominator and attn@V ops.
  - Softmax denominator via all-ones stationary matmul (PSUM-accumulated
    across k tiles), normalization as reciprocal*mul on DVE.
  - Phase B is software-pipelined: scores of tile i+1 are emitted between
    scores(i) and denom/attnV(i) so the mask->exp chain latency is hidden;
    the first two score groups of the next q-block are peeled ahead of the
    out-projection to cover the normalization latency.
  - Out-projection runs in bf16 (stationary ot tile, moving Wo), fp32 PSUM.
  - Weights/x are loaded with batched 3D-AP DMAs spread over the sync,
    gpsimd, scalar and tensor queues; y is stored as 1 MB row blocks.
"""

import sys

for _p in ("/opt/trn_rl_repo", "/root/.axon_site/_ro/trn_rl_repo"):
    if _p not in sys.path:
        sys.path.append(_p)

import numpy as np
from contextlib import ExitStack

import concourse.bass as bass
import concourse.bacc as bacc
import concourse.tile as tile
import concourse.mybir as mybir
from concourse.bass_utils import run_bass_kernel_spmd

F32 = mybir.dt.float32
F32R = mybir.dt.float32r
BF16 = mybir.dt.bfloat16
U16 = mybir.dt.uint16

B, T, C = 2, 2048, 2048
N_HEADS, N_KV_HEADS, HD = 16, 4, 128
G = N_HEADS // N_KV_HEADS  # heads per group = 4
GW = G * HD  # 512, per-group Q width / Wo row count
N_CORES = 8
TC = 512  # q-block width
NTC = T // TC  # 4
NCC = C // 128  # 16 contraction chunks
MASK_NEG = -1.0e30

_prog_cache = {}


def _build_program():
    nc = bacc.Bacc(
        "TRN2",
        target_bir_lowering=False,
        debug=False,
        enable_asserts=False,
        num_devices=N_CORES,
    )

    # host-pre-tiled layouts: contiguous >=2KB-per-partition lines so the
    # DMA descriptors stay fat (HBM small-descriptor penalty)
    xT = nc.dram_tensor("xT", [NTC * 4, 128, 4 * TC], F32, kind="ExternalInput").ap()
    wq = nc.dram_tensor("wq", [4, 128, 4 * GW], F32, kind="ExternalInput").ap()
    wk = nc.dram_tensor("wk", [128, NCC * HD], F32, kind="ExternalInput").ap()
    wv = nc.dram_tensor("wv", [128, NCC * HD], F32, kind="ExternalInput").ap()
    wo = nc.dram_tensor("wo", [GW, C], U16, kind="ExternalInput").ap()  # bf16 bits
    cos = nc.dram_tensor("cos", [HD, T], U16, kind="ExternalInput").ap()  # bf16 bits
    sin = nc.dram_tensor("sin", [HD, T], U16, kind="ExternalInput").ap()  # bf16 bits
    masks = nc.dram_tensor("masks", [128, TC], F32, kind="ExternalInput").ap()
    ident = nc.dram_tensor("ident", [128, 128], F32, kind="ExternalInput").ap()
    onesfull = nc.dram_tensor("onesfull", [128, 128], F32, kind="ExternalInput").ap()
    y = nc.dram_tensor("y", [T, C], F32, kind="ExternalOutput").ap()

    with tile.TileContext(nc) as tc, ExitStack() as ctx:
        sb = ctx.enter_context(tc.tile_pool(name="sb", bufs=1))
        xin = ctx.enter_context(tc.tile_pool(name="xin", bufs=2))
        rp = ctx.enter_context(tc.tile_pool(name="rp", bufs=2))
        ptp = ctx.enter_context(tc.tile_pool(name="ptp", bufs=6))
        nrm = ctx.enter_context(tc.tile_pool(name="nrm", bufs=2))
        otp = ctx.enter_context(tc.tile_pool(name="otp", bufs=2))
        ysb = ctx.enter_context(tc.tile_pool(name="ysb", bufs=2))
        ps = ctx.enter_context(tc.tile_pool(name="ps", bufs=1, space="PSUM"))

        # persistent activations
        qt_sb = sb.tile([128, G, T], F32R)  # [d, head, t]
        kt_sb = sb.tile([128, T], F32R)
        v_sb = sb.tile([128, T // 128, HD], F32R)  # [t-part, kt, d]

        # weights / tables. wq is split into four tiles (and wk/wv into
        # head/tail) so a matmul's DMA-completion dependency covers only
        # the chunks it actually reads.
        wq_t = [sb.tile([128, 4, GW], F32R, name=f"wq_t{i}") for i in range(4)]
        wk_a = sb.tile([128, 4, HD], F32R)
        wk_b = sb.tile([128, NCC - 4, HD], F32R)
        wv_a = sb.tile([128, 4, HD], F32R)
        wv_b = sb.tile([128, NCC - 4, HD], F32R)
        cos_sb = sb.tile([HD, T], BF16)
        sin_sb = sb.tile([HD, T], BF16)
        ident_sb = sb.tile([128, 128], F32)
        mask_sb = sb.tile([128, TC], F32)
        ones_sb = sb.tile([128, 128], F32R)
        wo_sb = sb.tile([128, G, C], BF16)

        # ---- weight prefetch. The two HW DMA rings (sync=SP, scalar=ACT)
        # serialize transfers per ring, so split the load: sync feeds x (+
        # the first K/V chunks), scalar feeds wq/wk/wv tails and tables.
        # wq group 0 goes as four per-ci chunks (fast first matmul), the
        # rest as contiguous 1MB blocks.
        # Keep the pre-loop DMA count at <=8: each matmul's completion wait
        # is on one of 8 shared lanes and counts every earlier-emitted DMA
        # on that lane, so extra prefetches here delay the FIRST matmul.
        for cc in range(4):
            nc.scalar.dma_start(
                wq_t[0][:, cc, :], wq[0, :, cc * GW : (cc + 1) * GW].bitcast(F32R)
            )
        nc.scalar.dma_start(wq_t[1][:].rearrange("p c j -> p (c j)"), wq[1].bitcast(F32R))
        nc.scalar.dma_start(
            wk_b[:].rearrange("p c j -> p (c j)"), wk[:, 4 * HD :].bitcast(F32R)
        )
        nc.scalar.dma_start(
            wv_b[:].rearrange("p c j -> p (c j)"), wv[:, 4 * HD :].bitcast(F32R)
        )
        nc.sync.dma_start(ident_sb[:], ident)

        # ---------------- phase A: projections + rope ----------------
        for tci in range(NTC):
            ts = slice(tci * TC, (tci + 1) * TC)
            qt_ps = [
                ps.tile([128, TC], F32, tag="q", bufs=4, name=f"qtps{tci}_{j}")
                for j in range(G)
            ]
            kt_ps = ps.tile([128, TC], F32, tag="kv", bufs=2, name=f"ktps{tci}")
            vt_ps = ps.tile([128, TC], F32, tag="kv", bufs=2, name=f"vtps{tci}")
            for cq in range(4):
                x_t = xin.tile([128, 4 * TC], F32R, tag="x", name=f"x{tci}_{cq}")
                blk = tci * 4 + cq
                # quads alternate sync / gpsimd rings (full SDMA transfer
                # bandwidth either way; keeps DMA issues out of the scalar
                # queue, which carries latency-critical copies)
                xq = nc.sync if cq % 2 == 0 else nc.gpsimd
                if tci == 0 and cq == 0:
                    # split the very first block so the first matmul can
                    # start after 512KB instead of 1MB
                    nc.sync.dma_start(
                        x_t[:, 0 : 2 * TC], xT[blk, :, 0 : 2 * TC].bitcast(F32R)
                    )
                    nc.sync.dma_start(
                        x_t[:, 2 * TC :], xT[blk, :, 2 * TC :].bitcast(F32R)
                    )
                    # first K/V weight chunks early on the sync ring
                    nc.sync.dma_start(
                        wk_a[:].rearrange("p c j -> p (c j)"),
                        wk[:, 0 : 4 * HD].bitcast(F32R),
                    )
                    nc.sync.dma_start(
                        wv_a[:].rearrange("p c j -> p (c j)"),
                        wv[:, 0 : 4 * HD].bitcast(F32R),
                    )
                else:
                    xq.dma_start(x_t[:], xT[blk].bitcast(F32R))
                if tci == 0 and cq == 1:
                    nc.scalar.dma_start(
                        wq_t[2][:].rearrange("p c j -> p (c j)"), wq[2].bitcast(F32R)
                    )
                    nc.sync.dma_start(
                        wq_t[3][:].rearrange("p c j -> p (c j)"), wq[3].bitcast(F32R)
                    )
                if tci == 0 and cq == 2:
                    nc.scalar.dma_start(cos_sb[:], cos.bitcast(BF16))
                    nc.scalar.dma_start(sin_sb[:], sin.bitcast(BF16))
                if tci == 0 and cq == 3:
                    nc.scalar.dma_start(mask_sb[:], masks)
                    nc.scalar.dma_start(ones_sb[:], onesfull.bitcast(F32R))
                    for h in range(G):
                        nc.scalar.dma_start(
                            wo_sb[:, h, :],
                            wo[h * 128 : (h + 1) * 128, :].bitcast(BF16),
                        )
                for sub in range(4):
                    ci = 4 * cq + sub
                    xs = x_t[:, sub * TC : (sub + 1) * TC]
                    st, sp = (ci == 0), (ci == NCC - 1)
                    wk_c = wk_a[:, ci, :] if ci < 4 else wk_b[:, ci - 4, :]
                    wv_c = wv_a[:, ci, :] if ci < 4 else wv_b[:, ci - 4, :]
                    for j in range(G):
                        nc.tensor.matmul(
                            qt_ps[j][:],
                            wq_t[ci // 4][:, ci % 4, j * HD : (j + 1) * HD],
                            xs,
                            start=st,
                            stop=sp,
                        )
                    nc.tensor.matmul(kt_ps[:], wk_c, xs, start=st, stop=sp)
                    nc.tensor.matmul(vt_ps[:], wv_c, xs, start=st, stop=sp)

            # rope: qt = q*cos + swap_half(q)*sin_signed. First evacuate all
            # six PSUM banks with fast aligned copies split across the
            # scalar and vector engines (so the next tci's matmuls aren't
            # gated on the slow rope chain), then build the half-swaps with
            # cross-partition scalar copies and finish the muls/add on DVE.
            q_raws = [
                rp.tile([128, TC], F32, tag=f"qraw{j}", name=f"qraw{tci}_{j}")
                for j in range(G)
            ]
            k_raw = rp.tile([128, TC], F32, tag="kraw", name=f"kraw{tci}")
            vt_f = rp.tile([128, TC], F32, tag="vtf", name=f"vtf{tci}")
            nc.scalar.copy(q_raws[0][:], qt_ps[0][:])
            nc.vector.tensor_copy(q_raws[1][:], qt_ps[1][:])
            nc.scalar.copy(q_raws[2][:], qt_ps[2][:])
            nc.vector.tensor_copy(q_raws[3][:], qt_ps[3][:])
            nc.scalar.copy(k_raw[:], kt_ps[:])
            nc.vector.tensor_copy(vt_f[:], vt_ps[:])

            # V: PE-transpose [d, t] -> [t, d] before the rope DVE work
            for s in range(TC // 128):
                kt_i = tci * (TC // 128) + s
                tp_t = ps.tile([128, TC], F32, tag="sp", bufs=2, name=f"tp{kt_i}")
                nc.tensor.transpose(
                    tp_t[:, 0:128], vt_f[:, s * 128 : (s + 1) * 128], ident_sb[:]
                )
                nc.scalar.copy(v_sb[:, kt_i, :], tp_t[:, 0:128])

            def make_rope(ts, q_raws, k_raw, tci):
                def rope(q_raw, dst, idx):
                    qsw = rp.tile([128, TC], F32, tag="qsw", name=f"qsw{idx}")
                    nc.scalar.copy(qsw[0:64, :], q_raw[64:128, :])
                    nc.scalar.copy(qsw[64:128, :], q_raw[0:64, :])
                    t1 = rp.tile([128, TC], F32, tag="t1", name=f"t1_{idx}")
                    nc.vector.tensor_mul(t1[:], q_raw[:], cos_sb[:, ts])
                    t2 = rp.tile([128, TC], F32, tag="t2", name=f"t2_{idx}")
                    nc.vector.tensor_mul(t2[:], qsw[:], sin_sb[:, ts])
                    nc.vector.tensor_add(dst, t1[:], t2[:])

                units = [
                    (lambda j=j: rope(q_raws[j], qt_sb[:, j, ts], f"{tci}_{j}"))
                    for j in range(G)
                ]
                units.append(lambda: rope(k_raw, kt_sb[:, ts], f"k{tci}"))
                return units

            rope_units = make_rope(ts, q_raws, k_raw, tci)
            if tci < NTC - 1:
                for u in rope_units:
                    u()
            else:
                # defer the last tci's rope (its outputs are only needed by
                # the later q-blocks) and spread it through qb0's pipeline
                # so the first q-block isn't queued behind 15 DVE ops
                deferred_rope = rope_units

        # -------- phase B: attention + out-projection per q-block --------
        # pending[(hg,)] closures emitted with a software-pipeline distance
        # of 2 between the score group S(i) and its denom/attnV group D(i).
        def make_block(qb, hg):
            # diag tiles first so PSUM start flag covers full columns
            kts = list(range(4 * qb, 4 * qb + 4)) + list(range(0, 4 * qb))
            n = len(kts)
            sb_ps = [
                ps.tile([128, TC], F32, tag="sp", bufs=2, name=f"sps{qb}_{hg}_{i}")
                for i in range(2)
            ]
            ot_ps = [
                ps.tile([128, TC], F32, tag="kv", bufs=2, name=f"otps{qb}_{hg}_{i}")
                for i in range(2)
            ]
            pts = {}

            def S(i):
                kt = kts[i]
                dj = kt - 4 * qb
                f0 = max(dj, 0) * 128
                cur = []
                for hh in range(2):
                    h = 2 * hg + hh
                    s_t = ps.tile(
                        [128, TC], F32, tag="q", bufs=4, name=f"st{qb}_{kt}_{h}"
                    )
                    if dj >= 0:
                        # pre-fill mask+zeros into PSUM off the critical
                        # chain; the score matmul accumulates on top
                        nc.vector.tensor_copy(
                            s_t[:, f0:TC], mask_sb[:, 0 : TC - f0]
                        )
                    nc.tensor.matmul(
                        s_t[:, f0:TC],
                        kt_sb[:, kt * 128 : (kt + 1) * 128],
                        qt_sb[:, h, qb * TC + f0 : (qb + 1) * TC],
                        start=(dj < 0),
                        stop=True,
                        skip_group_check=(dj >= 0),
                    )
                    pt = ptp.tile([128, TC], F32R, tag="pt", name=f"pt{qb}_{kt}_{h}")
                    nc.scalar.activation(
                        pt[:, f0:TC], s_t[:, f0:TC], mybir.ActivationFunctionType.Exp
                    )
                    cur.append(pt)
                pts[i] = (cur, f0)

            def D(i):
                kt = kts[i]
                cur, f0 = pts.pop(i)
                st_, sp_ = (i == 0), (i == n - 1)
                for hh in range(2):
                    nc.tensor.matmul(
                        sb_ps[hh][:, f0:TC],
                        ones_sb[:],
                        cur[hh][:, f0:TC],
                        start=st_,
                        stop=sp_,
                    )
                    nc.tensor.matmul(
                        ot_ps[hh][:, f0:TC],
                        v_sb[:, kt, :],
                        cur[hh][:, f0:TC],
                        start=st_,
                        stop=sp_,
                    )

            return S, D, n, sb_ps, ot_ps

        blocks = {}

        def emit_block(qb, hg, peeled, dve_filler=(), drain=True):
            S, D, n, sb_ps, ot_ps = blocks[(qb, hg)]
            filler = list(dve_filler)

            def fill():
                if filler:
                    filler.pop(0)()

            for i in range(peeled, n):
                S(i)
                if i >= 2:
                    D(i - 2)
                if i >= 1:
                    fill()
            D(n - 2)
            fill()
            D(n - 1)
            if drain:
                while filler:
                    filler.pop(0)()
            return sb_ps, ot_ps, filler

        # peel qb0's first two score groups so the PE rolls straight from
        # phase A into scores; the deferred rope rides along as filler
        blocks[(0, 0)] = make_block(0, 0)
        S0, _, _, _, _ = blocks[(0, 0)]
        S0(0)
        S0(1)

        peeled_next = 2
        for qb in range(NTC):
            blocks[(qb, 1)] = make_block(qb, 1)
            peeled = peeled_next

            # per-head normalized attention outputs (separate tiles so the
            # out-projection's h-loop tracks each head's norm individually)
            ot_t = [
                otp.tile([128, TC], BF16, tag=f"ot{h}", bufs=2, name=f"ot{qb}_{h}")
                for h in range(G)
            ]

            def norm_ops(hg, sbp, otps):
                rfs = {}

                def mk(hh):
                    h = 2 * hg + hh

                    def op_r():
                        r_f = nrm.tile([128, TC], F32, tag="rf", name=f"rf{qb}_{h}")
                        nc.vector.reciprocal_approx_fast(r_f[:], sbp[hh][:])
                        rfs[h] = r_f

                    def op_m():
                        nc.vector.tensor_mul(ot_t[h][:], otps[hh][:], rfs[h][:])

                    return [op_r, op_m]

                return mk(0) + mk(1)

            hg0_filler = list(deferred_rope) if qb == 0 else []
            sb0, ot0, left = emit_block(qb, 0, peeled, hg0_filler, drain=False)
            # hg0's normalization (and any leftover deferred rope) rides
            # along inside hg1's pipeline
            sb1, ot1, _ = emit_block(qb, 1, 0, left + norm_ops(0, sb0, ot0))

            # peel the first score groups of the next q-block ahead of the
            # normalization + out-projection; its first denominator/attnV
            # groups and two more score groups are interleaved into the
            # out-projection stream below
            if qb + 1 < NTC:
                blocks[(qb + 1, 0)] = make_block(qb + 1, 0)
                Sn, Dn, _, _, _ = blocks[(qb + 1, 0)]
                Sn(0)
                Sn(1)
                peeled_next = 4
            else:
                Sn = Dn = None
            for op in norm_ops(1, sb1, ot1):
                op()

            # out-projection for this q-block, stored as 512KB half-row
            # blocks alternating between the two DMA rings
            gi = 0
            for tl in range(TC // 128):
                tsub = qb * (TC // 128) + tl
                last = tsub == T // 128 - 1
                for half in range(2):
                    y_sb = ysb.tile([128, 2 * TC], F32, tag="y", name=f"ysb{tsub}_{half}")
                    for sub in range(2):
                        cc = 2 * half + sub
                        y_ps = ps.tile(
                            [128, TC], F32, tag="q", bufs=4, name=f"yps{tsub}_{cc}"
                        )
                        for h in range(G):
                            nc.tensor.matmul(
                                y_ps[:],
                                ot_t[h][:, tl * 128 : (tl + 1) * 128],
                                wo_sb[:, h, cc * TC : (cc + 1) * TC],
                                start=(h == 0),
                                stop=(h == G - 1),
                            )
                        nc.vector.tensor_copy(y_sb[:, sub * TC : (sub + 1) * TC], y_ps[:])
                        if last:
                            yq = (nc.sync, nc.scalar)[sub]
                            yq.dma_start(
                                y[
                                    tsub * 128 : (tsub + 1) * 128,
                                    cc * TC : (cc + 1) * TC,
                                ],
                                y_sb[:, sub * TC : (sub + 1) * TC],
                            )
                        if Sn is not None:
                            if gi == 3:
                                Dn(0)
                            elif gi == 7:
                                Dn(1)
                            elif gi == 11:
                                Sn(2)
                            elif gi == 13:
                                Sn(3)
                        gi += 1
                    if not last:
                        yq = nc.sync if half == 0 else nc.scalar
                        yq.dma_start(
                            y[
                                tsub * 128 : (tsub + 1) * 128,
                                half * 2 * TC : (half + 1) * 2 * TC,
                            ],
                            y_sb[:],
                        )

    nc.compile()
    return nc


def _rope_tables():
    theta = 1.0 / (10000.0 ** (np.arange(0, HD, 2, dtype=np.float32) / HD))
    freqs = np.arange(T, dtype=np.float32)[:, None] * theta[None, :]  # [T, 64]
    cos = np.concatenate([np.cos(freqs), np.cos(freqs)], axis=-1)  # [T, 128]
    sin = np.concatenate([np.sin(freqs), np.sin(freqs)], axis=-1)
    cosT = np.ascontiguousarray(cos.T).astype(np.float32)  # [128, T]
    sinT = np.ascontiguousarray(sin.T).astype(np.float32)
    sign = np.where(np.arange(HD) < HD // 2, np.float32(-1.0), np.float32(1.0))[:, None]
    sinT_signed = (sinT * sign).astype(np.float32)
    return cosT, sinT_signed


def _masks():
    # [128, TC]: first 128 cols = causal triangle bias (0 valid / -1e30
    # masked), rest zeros; copied into PSUM ahead of diagonal score matmuls
    p = np.arange(128)[:, None]
    f = np.arange(128)[None, :]
    tri = np.where(p <= f, 0.0, MASK_NEG).astype(np.float32)
    out = np.zeros((128, TC), dtype=np.float32)
    out[:, :128] = tri
    return out


def _to_bf16_bits(a):
    # round-to-nearest-even fp32 -> bf16, returned as uint16 bit pattern
    u = np.asarray(a, dtype=np.float32).view(np.uint32).astype(np.uint64)
    rounded = (u + 0x7FFF + ((u >> 16) & 1)) >> 16
    return rounded.astype(np.uint16)


def make_in_maps(x, Wq, Wk, Wv, Wo):
    x = np.asarray(x, dtype=np.float32)
    Wq = np.asarray(Wq, dtype=np.float32)
    Wk = np.asarray(Wk, dtype=np.float32)
    Wv = np.asarray(Wv, dtype=np.float32)
    Wo = np.asarray(Wo, dtype=np.float32)

    cosT, sinT = _rope_tables()
    masks = _masks()
    qscale = np.float32(1.0 / np.sqrt(HD))
    ident = np.eye(128, dtype=np.float32)
    onesfull = np.ones((128, 128), dtype=np.float32)

    def tile_x(xb):
        # xT [C, T] -> [tci*4+q, 128, 4*512] with 8KB contiguous lines
        xT_ = xb.T.reshape(4, 4, 128, 4, 512)  # [q, cc, p, tci, tt]
        return np.ascontiguousarray(xT_.transpose(3, 0, 2, 1, 4)).reshape(16, 128, 2048)

    def tile_wq(w):
        w_ = w.reshape(4, 4, 128, GW)  # [g, cc, p, j]
        return np.ascontiguousarray(w_.transpose(0, 2, 1, 3)).reshape(4, 128, 4 * GW)

    def tile_wkv(w):
        w_ = w.reshape(NCC, 128, HD)  # [c, p, j]
        return np.ascontiguousarray(w_.transpose(1, 0, 2)).reshape(128, NCC * HD)

    in_maps = []
    for c in range(N_CORES):
        b, g = divmod(c, N_KV_HEADS)
        in_maps.append(
            {
                "xT": tile_x(x[b]),
                "wq": tile_wq(
                    np.ascontiguousarray(Wq[:, g * GW : (g + 1) * GW]) * qscale
                ),
                "wk": tile_wkv(np.ascontiguousarray(Wk[:, g * HD : (g + 1) * HD])),
                "wv": tile_wkv(np.ascontiguousarray(Wv[:, g * HD : (g + 1) * HD])),
                "wo": _to_bf16_bits(Wo[g * GW : (g + 1) * GW, :]),
                "cos": _to_bf16_bits(cosT),
                "sin": _to_bf16_bits(sinT),
                "masks": masks,
                "ident": ident,
                "onesfull": onesfull,
            }
        )
    return in_maps


def kernel(x, Wq, Wk, Wv, Wo):
    if "nc" not in _prog_cache:
        _prog_cache["nc"] = _build_program()
    nc = _prog_cache["nc"]

    in_maps = make_in_maps(x, Wq, Wk, Wv, Wo)
    res = run_bass_kernel_spmd(nc, in_maps, list(range(N_CORES)))
    _prog_cache["last_results"] = res

    out = np.zeros((B, T, C), dtype=np.float32)
    for c in range(N_CORES):
        b = c // N_KV_HEADS
        out[b] += res.results[c]["y"]
    return out


# revision 56
# speedup vs baseline: 1.0749x; 1.0083x over previous
"""Causal self-attention (GQA + RoPE) Trainium2 kernel, v2.

Full-input contract: kernel(**inputs) takes the unsharded tensors and returns
the full [B, T, C] output. Internally shards over 8 NeuronCores as
(batch b in {0,1}) x (kv-head group g in {0..3}); each core computes the
attention output of its 4 query heads (one kv head) for its batch and the
partial out-projection against its 512 rows of Wo. The host sums the 4 group
partials per batch.

v2 structure (single TileContext scope, no mid-kernel barrier):
  - PSUM is one pool with shared tags across phases: "q" (4 bufs: Q-proj,
    score tiles, out-proj), "kv" (2: K/V-proj, attn@V accum), "sp" (2:
    V-transpose scratch, softmax denominators).
  - RoPE reads the projection PSUM directly on DVE, using cross-partition
    operand slices for the rotate-half term (PSUM in0 may have a different
    base partition than the SBUF in1 - verified on HW).
  - Causal F-restriction on score, exp, denominator and attn@V ops.
  - Softmax denominator via all-ones stationary matmul (PSUM-accumulated
    across k tiles), normalization as reciprocal*mul on DVE.
  - Phase B is software-pipelined: scores of tile i+1 are emitted between
    scores(i) and denom/attnV(i) so the mask->exp chain latency is hidden;
    the first two score groups of the next q-block are peeled ahead of the
    out-projection to cover the normalization latency.
  - Out-projection runs in bf16 (stationary ot tile, moving Wo), fp32 PSUM.
  - Weights/x are loaded with batched 3D-AP DMAs spread over the sync,
    gpsimd, scalar and tensor queues; y is stored as 1 MB row blocks.
"""

import sys

for _p in ("/opt/trn_rl_repo", "/root/.axon_site/_ro/trn_rl_repo"):
    if _p not in sys.path:
        sys.path.append(_p)

import numpy as np
from contextlib import ExitStack

import concourse.bass as bass
import concourse.bacc as bacc
import concourse.tile as tile
import concourse.mybir as mybir
from concourse.bass_utils import run_bass_kernel_spmd

F32 = mybir.dt.float32
F32R = mybir.dt.float32r
BF16 = mybir.dt.bfloat16
U16 = mybir.dt.uint16

B, T, C = 2, 2048, 2048
N_HEADS, N_KV_HEADS, HD = 16, 4, 128
G = N_HEADS // N_KV_HEADS  # heads per group = 4
GW = G * HD  # 512, per-group Q width / Wo row count
N_CORES = 8
TC = 512  # q-block width
NTC = T // TC  # 4
NCC = C // 128  # 16 contraction chunks
MASK_NEG = -1.0e30

_prog_cache = {}


def _build_program():
    nc = bacc.Bacc(
        "TRN2",
        target_bir_lowering=False,
        debug=False,
        enable_asserts=False,
        num_devices=N_CORES,
    )

    # host-pre-tiled layouts: contiguous >=2KB-per-partition lines so the
    # DMA descriptors stay fat (HBM small-descriptor penalty)
    xT = nc.dram_tensor("xT", [NTC * 4, 128, 4 * TC], F32, kind="ExternalInput").ap()
    wq = nc.dram_tensor("wq", [4, 128, 4 * GW], F32, kind="ExternalInput").ap()
    wq0s = nc.dram_tensor("wq0s", [4 * 128, GW], F32, kind="ExternalInput").ap()
    wk = nc.dram_tensor("wk", [128, NCC * HD], F32, kind="ExternalInput").ap()
    wv = nc.dram_tensor("wv", [128, NCC * HD], F32, kind="ExternalInput").ap()
    wk0 = nc.dram_tensor("wk0", [128, 4 * HD], F32, kind="ExternalInput").ap()
    wv0 = nc.dram_tensor("wv0", [128, 4 * HD], F32, kind="ExternalInput").ap()
    wo = nc.dram_tensor("wo", [GW, C], U16, kind="ExternalInput").ap()  # bf16 bits
    cos = nc.dram_tensor("cos", [HD, T], U16, kind="ExternalInput").ap()  # bf16 bits
    sin = nc.dram_tensor("sin", [HD, T], U16, kind="ExternalInput").ap()  # bf16 bits
    masks = nc.dram_tensor("masks", [128, TC], F32, kind="ExternalInput").ap()
    ident = nc.dram_tensor("ident", [128, 128], F32, kind="ExternalInput").ap()
    onesfull = nc.dram_tensor("onesfull", [128, 128], F32, kind="ExternalInput").ap()
    y = nc.dram_tensor("y", [T, C], F32, kind="ExternalOutput").ap()

    with tile.TileContext(nc) as tc, ExitStack() as ctx:
        sb = ctx.enter_context(tc.tile_pool(name="sb", bufs=1))
        xin = ctx.enter_context(tc.tile_pool(name="xin", bufs=2))
        rp = ctx.enter_context(tc.tile_pool(name="rp", bufs=2))
        ptp = ctx.enter_context(tc.tile_pool(name="ptp", bufs=6))
        nrm = ctx.enter_context(tc.tile_pool(name="nrm", bufs=2))
        otp = ctx.enter_context(tc.tile_pool(name="otp", bufs=2))
        ysb = ctx.enter_context(tc.tile_pool(name="ysb", bufs=2))
        ps = ctx.enter_context(tc.tile_pool(name="ps", bufs=1, space="PSUM"))

        # persistent activations
        qt_sb = sb.tile([128, G, T], F32R)  # [d, head, t]
        kt_sb = sb.tile([128, T], F32R)
        v_sb = sb.tile([128, T // 128, HD], F32R)  # [t-part, kt, d]

        # weights / tables. wq is split into four tiles (and wk/wv into
        # head/tail) so a matmul's DMA-completion dependency covers only
        # the chunks it actually reads.
        wq_t = [sb.tile([128, 4, GW], F32R, name=f"wq_t{i}") for i in range(4)]
        wk_a = sb.tile([128, 4, HD], F32R)
        wk_b = sb.tile([128, NCC - 4, HD], F32R)
        wv_a = sb.tile([128, 4, HD], F32R)
        wv_b = sb.tile([128, NCC - 4, HD], F32R)
        cos_sb = sb.tile([HD, T], BF16)
        sin_sb = sb.tile([HD, T], BF16)
        ident_sb = sb.tile([128, 128], F32)
        mask_sb = sb.tile([128, TC], F32)
        ones_sb = sb.tile([128, 128], F32R)
        wo_sb = sb.tile([128, G, C], BF16)

        # ---- weight prefetch. The two HW DMA rings (sync=SP, scalar=ACT)
        # serialize transfers per ring, so split the load: sync feeds x (+
        # the first K/V chunks), scalar feeds wq/wk/wv tails and tables.
        # wq group 0 goes as four per-ci chunks (fast first matmul), the
        # rest as contiguous 1MB blocks.
        # Keep the pre-loop DMA count at <=8: each matmul's completion wait
        # is on one of 8 shared lanes and counts every earlier-emitted DMA
        # on that lane, so extra prefetches here delay the FIRST matmul.
        for cc in range(4):
            nc.scalar.dma_start(
                wq_t[0][:, cc, :],
                wq0s[cc * 128 : (cc + 1) * 128, :].bitcast(F32R),
            )
        nc.scalar.dma_start(wq_t[1][:].rearrange("p c j -> p (c j)"), wq[1].bitcast(F32R))
        nc.scalar.dma_start(
            wk_b[:].rearrange("p c j -> p (c j)"), wk[:, 4 * HD :].bitcast(F32R)
        )
        nc.scalar.dma_start(
            wv_b[:].rearrange("p c j -> p (c j)"), wv[:, 4 * HD :].bitcast(F32R)
        )
        nc.sync.dma_start(ident_sb[:], ident)

        # ---------------- phase A: projections + rope ----------------
        for tci in range(NTC):
            ts = slice(tci * TC, (tci + 1) * TC)
            qt_ps = [
                ps.tile([128, TC], F32, tag="q", bufs=4, name=f"qtps{tci}_{j}")
                for j in range(G)
            ]
            kt_ps = ps.tile([128, TC], F32, tag="kv", bufs=2, name=f"ktps{tci}")
            vt_ps = ps.tile([128, TC], F32, tag="kv", bufs=2, name=f"vtps{tci}")
            for cq in range(4):
                x_t = xin.tile([128, 4 * TC], F32R, tag="x", name=f"x{tci}_{cq}")
                blk = tci * 4 + cq
                # quads alternate sync / gpsimd rings (full SDMA transfer
                # bandwidth either way; keeps DMA issues out of the scalar
                # queue, which carries latency-critical copies)
                xq = nc.sync if cq % 2 == 0 else nc.gpsimd
                if tci == 0 and cq == 0:
                    # split the very first block so the first matmul can
                    # start after 512KB instead of 1MB
                    nc.sync.dma_start(
                        x_t[:, 0 : 2 * TC], xT[blk, :, 0 : 2 * TC].bitcast(F32R)
                    )
                    nc.sync.dma_start(
                        x_t[:, 2 * TC :], xT[blk, :, 2 * TC :].bitcast(F32R)
                    )
                    # first K/V weight chunks early on the sync ring
                    nc.sync.dma_start(
                        wk_a[:].rearrange("p c j -> p (c j)"), wk0.bitcast(F32R)
                    )
                    nc.sync.dma_start(
                        wv_a[:].rearrange("p c j -> p (c j)"), wv0.bitcast(F32R)
                    )
                else:
                    xq.dma_start(x_t[:], xT[blk].bitcast(F32R))
                if tci == 0 and cq == 1:
                    nc.scalar.dma_start(
                        wq_t[2][:].rearrange("p c j -> p (c j)"), wq[2].bitcast(F32R)
                    )
                    nc.sync.dma_start(
                        wq_t[3][:].rearrange("p c j -> p (c j)"), wq[3].bitcast(F32R)
                    )
                if tci == 0 and cq == 2:
                    nc.scalar.dma_start(cos_sb[:], cos.bitcast(BF16))
                    nc.scalar.dma_start(sin_sb[:], sin.bitcast(BF16))
                if tci == 0 and cq == 3:
                    nc.scalar.dma_start(mask_sb[:], masks)
                    nc.scalar.dma_start(ones_sb[:], onesfull.bitcast(F32R))
                    for h in range(G):
                        nc.scalar.dma_start(
                            wo_sb[:, h, :],
                            wo[h * 128 : (h + 1) * 128, :].bitcast(BF16),
                        )
                for sub in range(4):
                    ci = 4 * cq + sub
                    xs = x_t[:, sub * TC : (sub + 1) * TC]
                    st, sp = (ci == 0), (ci == NCC - 1)
                    wk_c = wk_a[:, ci, :] if ci < 4 else wk_b[:, ci - 4, :]
                    wv_c = wv_a[:, ci, :] if ci < 4 else wv_b[:, ci - 4, :]
                    for j in range(G):
                        nc.tensor.matmul(
                            qt_ps[j][:],
                            wq_t[ci // 4][:, ci % 4, j * HD : (j + 1) * HD],
                            xs,
                            start=st,
                            stop=sp,
                        )
                    nc.tensor.matmul(kt_ps[:], wk_c, xs, start=st, stop=sp)
                    nc.tensor.matmul(vt_ps[:], wv_c, xs, start=st, stop=sp)

            # rope: qt = q*cos + swap_half(q)*sin_signed. First evacuate all
            # six PSUM banks with fast aligned copies split across the
            # scalar and vector engines (so the next tci's matmuls aren't
            # gated on the slow rope chain), then build the half-swaps with
            # cross-partition scalar copies and finish the muls/add on DVE.
            q_raws = [
                rp.tile([128, TC], F32, tag=f"qraw{j}", name=f"qraw{tci}_{j}")
                for j in range(G)
            ]
            k_raw = rp.tile([128, TC], F32, tag="kraw", name=f"kraw{tci}")
            vt_f = rp.tile([128, TC], F32, tag="vtf", name=f"vtf{tci}")
            nc.scalar.copy(q_raws[0][:], qt_ps[0][:])
            nc.vector.tensor_copy(q_raws[1][:], qt_ps[1][:])
            nc.scalar.copy(q_raws[2][:], qt_ps[2][:])
            nc.vector.tensor_copy(q_raws[3][:], qt_ps[3][:])
            nc.scalar.copy(k_raw[:], kt_ps[:])
            nc.vector.tensor_copy(vt_f[:], vt_ps[:])

            # V: PE-transpose [d, t] -> [t, d] before the rope DVE work
            for s in range(TC // 128):
                kt_i = tci * (TC // 128) + s
                tp_t = ps.tile([128, TC], F32, tag="sp", bufs=2, name=f"tp{kt_i}")
                nc.tensor.transpose(
                    tp_t[:, 0:128], vt_f[:, s * 128 : (s + 1) * 128], ident_sb[:]
                )
                nc.scalar.copy(v_sb[:, kt_i, :], tp_t[:, 0:128])

            def make_rope(ts, q_raws, k_raw, tci):
                def rope(q_raw, dst, idx):
                    qsw = rp.tile([128, TC], F32, tag="qsw", name=f"qsw{idx}")
                    nc.scalar.copy(qsw[0:64, :], q_raw[64:128, :])
                    nc.scalar.copy(qsw[64:128, :], q_raw[0:64, :])
                    t1 = rp.tile([128, TC], F32, tag="t1", name=f"t1_{idx}")
                    nc.vector.tensor_mul(t1[:], q_raw[:], cos_sb[:, ts])
                    t2 = rp.tile([128, TC], F32, tag="t2", name=f"t2_{idx}")
                    nc.vector.tensor_mul(t2[:], qsw[:], sin_sb[:, ts])
                    nc.vector.tensor_add(dst, t1[:], t2[:])

                units = [
                    (lambda j=j: rope(q_raws[j], qt_sb[:, j, ts], f"{tci}_{j}"))
                    for j in range(G)
                ]
                units.append(lambda: rope(k_raw, kt_sb[:, ts], f"k{tci}"))
                return units

            rope_units = make_rope(ts, q_raws, k_raw, tci)
            if tci < NTC - 1:
                for u in rope_units:
                    u()
            else:
                # defer the last tci's rope (its outputs are only needed by
                # the later q-blocks) and spread it through qb0's pipeline
                # so the first q-block isn't queued behind 15 DVE ops
                deferred_rope = rope_units

        # -------- phase B: attention + out-projection per q-block --------
        # pending[(hg,)] closures emitted with a software-pipeline distance
        # of 2 between the score group S(i) and its denom/attnV group D(i).
        def make_block(qb, hg):
            # diag tiles first so PSUM start flag covers full columns
            kts = list(range(4 * qb, 4 * qb + 4)) + list(range(0, 4 * qb))
            n = len(kts)
            sb_ps = [
                ps.tile([128, TC], F32, tag="sp", bufs=2, name=f"sps{qb}_{hg}_{i}")
                for i in range(2)
            ]
            ot_ps = [
                ps.tile([128, TC], F32, tag="kv", bufs=2, name=f"otps{qb}_{hg}_{i}")
                for i in range(2)
            ]
            pts = {}

            def S(i):
                kt = kts[i]
                dj = kt - 4 * qb
                f0 = max(dj, 0) * 128
                cur = []
                for hh in range(2):
                    h = 2 * hg + hh
                    s_t = ps.tile(
                        [128, TC], F32, tag="q", bufs=4, name=f"st{qb}_{kt}_{h}"
                    )
                    if dj >= 0:
                        # pre-fill mask+zeros into PSUM off the critical
                        # chain; the score matmul accumulates on top
                        nc.vector.tensor_copy(
                            s_t[:, f0:TC], mask_sb[:, 0 : TC - f0]
                        )
                    nc.tensor.matmul(
                        s_t[:, f0:TC],
                        kt_sb[:, kt * 128 : (kt + 1) * 128],
                        qt_sb[:, h, qb * TC + f0 : (qb + 1) * TC],
                        start=(dj < 0),
                        stop=True,
                        skip_group_check=(dj >= 0),
                    )
                    pt = ptp.tile([128, TC], F32R, tag="pt", name=f"pt{qb}_{kt}_{h}")
                    nc.scalar.activation(
                        pt[:, f0:TC], s_t[:, f0:TC], mybir.ActivationFunctionType.Exp
                    )
                    cur.append(pt)
                pts[i] = (cur, f0)

            def D(i):
                kt = kts[i]
                cur, f0 = pts.pop(i)
                st_, sp_ = (i == 0), (i == n - 1)
                for hh in range(2):
                    nc.tensor.matmul(
                        sb_ps[hh][:, f0:TC],
                        ones_sb[:],
                        cur[hh][:, f0:TC],
                        start=st_,
                        stop=sp_,
                    )
                    nc.tensor.matmul(
                        ot_ps[hh][:, f0:TC],
                        v_sb[:, kt, :],
                        cur[hh][:, f0:TC],
                        start=st_,
                        stop=sp_,
                    )

            return S, D, n, sb_ps, ot_ps

        blocks = {}

        def emit_block(qb, hg, peeled, dve_filler=(), drain=True):
            S, D, n, sb_ps, ot_ps = blocks[(qb, hg)]
            filler = list(dve_filler)

            def fill():
                if filler:
                    filler.pop(0)()

            for i in range(peeled, n):
                S(i)
                if i >= 2:
                    D(i - 2)
                if i >= 1:
                    fill()
            D(n - 2)
            fill()
            D(n - 1)
            if drain:
                while filler:
                    filler.pop(0)()
            return sb_ps, ot_ps, filler

        # peel qb0's first two score groups so the PE rolls straight from
        # phase A into scores; the deferred rope rides along as filler
        blocks[(0, 0)] = make_block(0, 0)
        S0, _, _, _, _ = blocks[(0, 0)]
        S0(0)
        S0(1)

        peeled_next = 2
        for qb in range(NTC):
            blocks[(qb, 1)] = make_block(qb, 1)
            peeled = peeled_next

            # per-head normalized attention outputs (separate tiles so the
            # out-projection's h-loop tracks each head's norm individually)
            ot_t = [
                otp.tile([128, TC], BF16, tag=f"ot{h}", bufs=2, name=f"ot{qb}_{h}")
                for h in range(G)
            ]

            def norm_ops(hg, sbp, otps):
                rfs = {}

                def mk(hh):
                    h = 2 * hg + hh

                    def op_r():
                        r_f = nrm.tile([128, TC], F32, tag="rf", name=f"rf{qb}_{h}")
                        nc.vector.reciprocal_approx_fast(r_f[:], sbp[hh][:])
                        rfs[h] = r_f

                    def op_m():
                        nc.vector.tensor_mul(ot_t[h][:], otps[hh][:], rfs[h][:])

                    return [op_r, op_m]

                return mk(0) + mk(1)

            hg0_filler = list(deferred_rope) if qb == 0 else []
            sb0, ot0, left = emit_block(qb, 0, peeled, hg0_filler, drain=False)
            # hg0's normalization (and any leftover deferred rope) rides
            # along inside hg1's pipeline
            sb1, ot1, _ = emit_block(qb, 1, 0, left + norm_ops(0, sb0, ot0))

            # peel the first score groups of the next q-block ahead of the
            # normalization + out-projection; its first denominator/attnV
            # groups and two more score groups are interleaved into the
            # out-projection stream below
            if qb + 1 < NTC:
                blocks[(qb + 1, 0)] = make_block(qb + 1, 0)
                Sn, Dn, _, _, _ = blocks[(qb + 1, 0)]
                Sn(0)
                Sn(1)
                peeled_next = 4
            else:
                Sn = Dn = None
            for op in norm_ops(1, sb1, ot1):
                op()

            # out-projection for this q-block, stored as 512KB half-row
            # blocks alternating between the two DMA rings
            gi = 0
            for tl in range(TC // 128):
                tsub = qb * (TC // 128) + tl
                last = tsub == T // 128 - 1
                for half in range(2):
                    y_sb = ysb.tile([128, 2 * TC], F32, tag="y", name=f"ysb{tsub}_{half}")
                    for sub in range(2):
                        cc = 2 * half + sub
                        y_ps = ps.tile(
                            [128, TC], F32, tag="q", bufs=4, name=f"yps{tsub}_{cc}"
                        )
                        for h in range(G):
                            nc.tensor.matmul(
                                y_ps[:],
                                ot_t[h][:, tl * 128 : (tl + 1) * 128],
                                wo_sb[:, h, cc * TC : (cc + 1) * TC],
                                start=(h == 0),
                                stop=(h == G - 1),
                            )
                        nc.vector.tensor_copy(y_sb[:, sub * TC : (sub + 1) * TC], y_ps[:])
                        if last:
                            yq = (nc.sync, nc.scalar)[sub]
                            yq.dma_start(
                                y[
                                    tsub * 128 : (tsub + 1) * 128,
                                    cc * TC : (cc + 1) * TC,
                                ],
                                y_sb[:, sub * TC : (sub + 1) * TC],
                            )
                        if Sn is not None:
                            if gi == 3:
                                Dn(0)
                            elif gi == 7:
                                Dn(1)
                            elif gi == 11:
                                Sn(2)
                            elif gi == 13:
                                Sn(3)
                        gi += 1
                    if not last:
                        yq = nc.sync if half == 0 else nc.scalar
                        yq.dma_start(
                            y[
                                tsub * 128 : (tsub + 1) * 128,
                                half * 2 * TC : (half + 1) * 2 * TC,
                            ],
                            y_sb[:],
                        )

    nc.compile()
    return nc


def _rope_tables():
    theta = 1.0 / (10000.0 ** (np.arange(0, HD, 2, dtype=np.float32) / HD))
    freqs = np.arange(T, dtype=np.float32)[:, None] * theta[None, :]  # [T, 64]
    cos = np.concatenate([np.cos(freqs), np.cos(freqs)], axis=-1)  # [T, 128]
    sin = np.concatenate([np.sin(freqs), np.sin(freqs)], axis=-1)
    cosT = np.ascontiguousarray(cos.T).astype(np.float32)  # [128, T]
    sinT = np.ascontiguousarray(sin.T).astype(np.float32)
    sign = np.where(np.arange(HD) < HD // 2, np.float32(-1.0), np.float32(1.0))[:, None]
    sinT_signed = (sinT * sign).astype(np.float32)
    return cosT, sinT_signed


def _masks():
    # [128, TC]: first 128 cols = causal triangle bias (0 valid / -1e30
    # masked), rest zeros; copied into PSUM ahead of diagonal score matmuls
    p = np.arange(128)[:, None]
    f = np.arange(128)[None, :]
    tri = np.where(p <= f, 0.0, MASK_NEG).astype(np.float32)
    out = np.zeros((128, TC), dtype=np.float32)
    out[:, :128] = tri
    return out


def _to_bf16_bits(a):
    # round-to-nearest-even fp32 -> bf16, returned as uint16 bit pattern
    u = np.asarray(a, dtype=np.float32).view(np.uint32).astype(np.uint64)
    rounded = (u + 0x7FFF + ((u >> 16) & 1)) >> 16
    return rounded.astype(np.uint16)


def make_in_maps(x, Wq, Wk, Wv, Wo):
    x = np.asarray(x, dtype=np.float32)
    Wq = np.asarray(Wq, dtype=np.float32)
    Wk = np.asarray(Wk, dtype=np.float32)
    Wv = np.asarray(Wv, dtype=np.float32)
    Wo = np.asarray(Wo, dtype=np.float32)

    cosT, sinT = _rope_tables()
    masks = _masks()
    qscale = np.float32(1.0 / np.sqrt(HD))
    ident = np.eye(128, dtype=np.float32)
    onesfull = np.ones((128, 128), dtype=np.float32)

    def tile_x(xb):
        # xT [C, T] -> [tci*4+q, 128, 4*512] with 8KB contiguous lines
        xT_ = xb.T.reshape(4, 4, 128, 4, 512)  # [q, cc, p, tci, tt]
        return np.ascontiguousarray(xT_.transpose(3, 0, 2, 1, 4)).reshape(16, 128, 2048)

    def tile_wq(w):
        w_ = w.reshape(4, 4, 128, GW)  # [g, cc, p, j]
        return np.ascontiguousarray(w_.transpose(0, 2, 1, 3)).reshape(4, 128, 4 * GW)

    def tile_wkv(w):
        w_ = w.reshape(NCC, 128, HD)  # [c, p, j]
        return np.ascontiguousarray(w_.transpose(1, 0, 2)).reshape(128, NCC * HD)

    in_maps = []
    for c in range(N_CORES):
        b, g = divmod(c, N_KV_HEADS)
        wq_s = np.ascontiguousarray(Wq[:, g * GW : (g + 1) * GW]) * qscale
        wk_t = tile_wkv(np.ascontiguousarray(Wk[:, g * HD : (g + 1) * HD]))
        wv_t = tile_wkv(np.ascontiguousarray(Wv[:, g * HD : (g + 1) * HD]))
        in_maps.append(
            {
                "xT": tile_x(x[b]),
                "wq": tile_wq(wq_s),
                "wq0s": np.ascontiguousarray(wq_s[: 4 * 128, :]),
                "wk": wk_t,
                "wv": wv_t,
                "wk0": np.ascontiguousarray(wk_t[:, : 4 * HD]),
                "wv0": np.ascontiguousarray(wv_t[:, : 4 * HD]),
                "wo": _to_bf16_bits(Wo[g * GW : (g + 1) * GW, :]),
                "cos": _to_bf16_bits(cosT),
                "sin": _to_bf16_bits(sinT),
                "masks": masks,
                "ident": ident,
                "onesfull": onesfull,
            }
        )
    return in_maps


def kernel(x, Wq, Wk, Wv, Wo):
    if "nc" not in _prog_cache:
        _prog_cache["nc"] = _build_program()
    nc = _prog_cache["nc"]

    in_maps = make_in_maps(x, Wq, Wk, Wv, Wo)
    res = run_bass_kernel_spmd(nc, in_maps, list(range(N_CORES)))
    _prog_cache["last_results"] = res

    out = np.zeros((B, T, C), dtype=np.float32)
    for c in range(N_CORES):
        b = c // N_KV_HEADS
        out[b] += res.results[c]["y"]
    return out
